# revision 15
# baseline (speedup 1.0000x reference)
"""Multi-head cross-attention (B=2, N=1024, L=4096, D=1024, H=16) on 8 trn2
NeuronCores.

Sharding: batch x head-group data/tensor parallel. Core c handles batch
c//4 and heads 4*(c%4) .. 4*(c%4)+3 (weight columns sliced per head group,
Wo row-sliced; partial outputs summed on the host during unsharding).

Math simplifications vs the reference (exact, not approximations):
  - bk dropped: scores shift per-query by (q+bq)@bk, softmax-invariant.
  - bv dropped on device: softmax rows sum to 1, so the bias contributes
    bv @ Wo, a constant row added on the host together with bo.
  - softmax scale folded into Wq and bq on the host.

Per-core device program. Projections run in fp32r (bitwise fp32, full
rate); the attention inner loop (scores, softmax weights, V) runs in
bf16 -- measured end-to-end error 1.8e-3, and the halved SBUF traffic
keeps the power governor from down-clocking the PE:
  qT = W.T @ x.T       (channels on partitions, head pairs stacked 64+64)
  kT likewise per 512-key block; v = x @ Wv (keys on partitions),
    augmented with a ones column and pre-multiplied by the pad-keep mask
  per (query-block, head-pair, keytile):
     sT[keys,q] = kT.T @ qT   (two row-paired K=64 bf16 matmuls)
     pT = exp(sT) in bf16     (one ACT op over both heads' banks)
     oT_aug[65,q] += va.T @ pT   (PSUM accumulation; row 64 = denominator)
  out_part = (oT/denom).T @ Wo_slice

Scheduling: PE clock is HAM-gated (1.2 GHz cold, 2.4 GHz after ~3.4us of
sustained activity) and power-throttled under sustained full draw, so the
emission order keeps the PE fed without idling ACT:
  - projections and combo (0,0) attention interleave with K/V staging in
    single-bank PSUM tiles (no drain ever blocks the next matmul)
  - in the attention-only phase a third score buffer (in the banks freed
    by the projection pool) lets two exps stay in flight so the ACT
    stream -- the throughput floor -- never starves
"""
import sys

sys.path.insert(0, "/opt/trn_rl_repo")

import numpy as np

import concourse.bass as bass
import concourse.tile as tile
from concourse import bacc, mybir
from concourse.bass_utils import run_bass_kernel_spmd

dt = mybir.dt
ts = bass.ts

B, N, L, D = 2, 1024, 4096, 1024
H, DH = 16, 64
HC = 4            # heads per core
CS = HC * DH      # 256 channel slice per core
SCALE = DH ** -0.5
N_CORES = 8
QB, KB = 2, 8     # query blocks of 512, key blocks of 512
DQC = 8           # contraction chunks of 128
KT = 32           # keytiles of 128

TRACE = False
LAST_EXEC_NS = None
_cache = {}


def _build():
    nc = bacc.Bacc("TRN2", target_bir_lowering=False, debug=False,
                   num_devices=N_CORES)

    f32r = dt.float32r
    xTq = nc.dram_tensor("xTq", [D, N], f32r, kind="ExternalInput").ap()
    xTkv = nc.dram_tensor("xTkv", [D, L], f32r, kind="ExternalInput").ap()
    wq = nc.dram_tensor("wq", [D, CS], f32r, kind="ExternalInput").ap()
    wk = nc.dram_tensor("wk", [D, CS], f32r, kind="ExternalInput").ap()
    wv = nc.dram_tensor("wv", [D, CS], f32r, kind="ExternalInput").ap()
    wo = nc.dram_tensor("wo", [CS, D], f32r, kind="ExternalInput").ap()
    bqp = nc.dram_tensor("bqp", [128, 2], dt.float32, kind="ExternalInput").ap()
    keep = nc.dram_tensor("keep", [128, KT, HC], dt.float32,
                          kind="ExternalInput").ap()
    out = nc.dram_tensor("out", [N, D], dt.float32, kind="ExternalOutput").ap()

    with tile.TileContext(nc) as tc:
        _emit(nc, tc, xTq, xTkv, wq, wk, wv, wo, bqp, keep, out)
    nc.compile()
    return nc


def _emit(nc, tc, xTq, xTkv, wq, wk, wv, wo, bqp, keep, out):
    import contextlib

    ctx = contextlib.ExitStack()
    with ctx:
        persist = ctx.enter_context(tc.tile_pool(name="persist", bufs=1))
        xr_pool = ctx.enter_context(tc.tile_pool(name="xr", bufs=8))
        pT_pool = ctx.enter_context(tc.tile_pool(name="pT", bufs=4))
        rb_pool = ctx.enter_context(tc.tile_pool(name="rbs", bufs=2))
        outsb_pool = ctx.enter_context(tc.tile_pool(name="outsb", bufs=3))
        psS = ctx.enter_context(tc.tile_pool(name="psS", bufs=2, space="PSUM"))
        psO = ctx.enter_context(tc.tile_pool(name="psO", bufs=2, space="PSUM"))
        psA_cm = tc.tile_pool(name="psA", bufs=2, space="PSUM")
        psA = psA_cm.__enter__()
        lp = nc.allow_low_precision(reason="fp32r/bf16 attention internals")
        lp.__enter__()

        # ---- weight loading: one DMA straight into an fp32r tile ---------
        def load_w3(name, src, d0, chunked=False):
            r = persist.tile([128, d0, src.shape[1]], dt.float32r, tag=name,
                             name=name)
            rs = src.rearrange("(c p) n -> p c n", p=128)
            if chunked:
                for c in range(d0):
                    nc.sync.dma_start(r[:, c], rs[:, c])
            else:
                nc.sync.dma_start(r[:], rs)
            return r

        wq_r = load_w3("wqr", wq, DQC, chunked=True)          # [128, 8, 256]
        bq_sb = persist.tile([128, 2], dt.float32, tag="bqp", name="bq_sb")
        nc.sync.dma_start(bq_sb[:], bqp)
        ones128_f = persist.tile([1, 128], dt.float32, tag="o128",
                                 name="ones128_f")
        nc.vector.memset(ones128_f[:], 1.0)
        ones128_r = ones128_f[:].bitcast(dt.float32r)

        # ---- persistent activation tiles (attention operands in bf16) ---
        qT_sb = [persist.tile([128, N], dt.bfloat16, tag=f"qT{cc}", name=f"qT{cc}")
                 for cc in range(2)]
        kT_sb = [[persist.tile([128, 512], dt.bfloat16, tag=f"kT{cc}_{kb}",
                               name=f"kT{cc}_{kb}") for kb in range(KB)]
                 for cc in range(2)]
        va_sb = [persist.tile([128, HC, 65], dt.bfloat16, tag=f"va{kt}",
                              name=f"va{kt}") for kt in range(KT)]
        onT_sb = [persist.tile([128, N], dt.float32r, tag=f"onT{cc}",
                               name=f"onT{cc}") for cc in range(2)]

        # ---- Q projection (into psS while attention hasn't started) -----
        xq_r = []
        for dq in range(DQC):
            xr = xr_pool.tile([128, N], dt.float32r, tag="xr", name=f"xq{dq}")
            nc.sync.dma_start(xr[:], xTq[ts(dq, 128), :])
            xq_r.append(xr)
        for cc in range(2):
            qp = psS.tile([128, N], dt.float32, tag="sp", name=f"qp{cc}")
            for qb in range(QB):
                for dq in range(DQC):
                    nc.tensor.matmul(qp[:, ts(qb, 512)],
                                     wq_r[:, dq, ts(cc, 128)],
                                     xq_r[dq][:, ts(qb, 512)],
                                     start=(dq == 0), stop=(dq == DQC - 1))
            nc.vector.tensor_scalar_add(qT_sb[cc][:], qp[:], bq_sb[:, cc:cc + 1])

        wk_r = load_w3("wkr", wk, DQC, chunked=True)
        wv_r = load_w3("wvr", wv, DQC, chunked=True)
        keep_f = persist.tile([128, KT, HC], dt.float32, tag="keepf",
                              name="keep_f")
        nc.sync.dma_start(keep_f[:], keep)

        # ---- attention helpers ------------------------------------------
        oPs = {}
        sp_pools = [psS]          # phase C appends the extra 2-bank pool
        sp_i = [0]

        def sp_tile(name):
            # rotate score tiles over psS's 2 bufs (+ spX in phase C:
            # psS, psS, spX, psS, psS, spX, ... = 3 exps in flight)
            if len(sp_pools) == 1:
                pool = sp_pools[0]
            else:
                pool = sp_pools[0] if sp_i[0] % 3 < 2 else sp_pools[1]
            sp_i[0] += 1
            return pool.tile([128, 1024], dt.float32, tag="sp", name=name)

        def open_oP(qb, hp):
            oPs[(qb, hp)] = [
                psO.tile([128, 512], dt.float32, tag="oP", name=f"oP{qb}{hp}{h}")
                for h in range(2)
            ]

        pTs = {}

        def attn_scores(qb, hp, kt):
            kb, kti = kt // 4, kt % 4
            sp = sp_tile(f"sp{qb}{hp}{kt}")
            for h in range(2):
                nc.tensor.matmul(
                    sp[:, ts(h, 512)],
                    kT_sb[hp][kb][ts(h, 64), ts(kti, 128)],
                    qT_sb[hp][ts(h, 64), ts(qb, 512)],
                    start=True, stop=True,
                )
            pT = pT_pool.tile([128, 1024], dt.bfloat16, tag="pT",
                              name=f"pT{qb}{hp}{kt}")
            nc.scalar.activation(pT[:], sp[:], mybir.ActivationFunctionType.Exp)
            pTs[(qb, hp, kt)] = pT

        def attn_v(qb, hp, kt):
            pT = pTs.pop((qb, hp, kt))
            oP = oPs[(qb, hp)]
            for h in range(2):
                nc.tensor.matmul(
                    oP[h][0:65, :], va_sb[kt][:, hp * 2 + h, :], pT[:, ts(h, 512)],
                    start=(kt == 0), stop=(kt == KT - 1),
                )

        LAG = 2

        def attn_kts(qb, hp, kts, final=False):
            # scores run LAG keytiles ahead of attnV so the in-order PE
            # queue never parks on an exp that hasn't finished
            for kt in kts:
                attn_scores(qb, hp, kt)
                if kt - LAG >= 0:
                    attn_v(qb, hp, kt - LAG)
            if final:
                for kt in range(KT - LAG, KT):
                    attn_v(qb, hp, kt)

        def attn_norm(qb, hp):
            oP = oPs.pop((qb, hp))
            for h in range(2):
                den = rb_pool.tile([1, 512], dt.float32, tag="den",
                                   name=f"den{qb}{hp}{h}")
                nc.vector.tensor_copy(den[:], oP[h][64:65, :])
                rdf = rb_pool.tile([1, 512], dt.float32, tag="rdf",
                                   name=f"rdf{qb}{hp}{h}")
                nc.vector.reciprocal_approx_fast(rdf[:], den[:])
                rd = rb_pool.tile([1, 512], dt.float32r, tag="rd",
                                  name=f"rd{qb}{hp}{h}")
                nc.vector.tensor_copy(rd[:], rdf[:])
                rb = sp_tile(f"rb{qb}{hp}{h}")[:, 0:512]
                nc.tensor.matmul(rb, ones128_r, rd[:], start=True, stop=True)
                rb_sb = rb_pool.tile([128, 512], dt.float32, tag="rbs",
                                     name=f"rbs{qb}{hp}{h}")
                nc.vector.tensor_copy(rb_sb[:], rb)
                nc.vector.tensor_mul(onT_sb[hp][ts(h, 64), ts(qb, 512)],
                                     oP[h][0:64, :], rb_sb[0:64, :])

        # ---- K/V projections interleaved with attention on (qb0, hp0) ---
        open_oP(0, 0)
        for kb in range(KB):
            xrs = []
            for dq in range(DQC):
                xr = xr_pool.tile([128, 512], dt.float32r, tag="xr",
                                  name=f"xk{kb}_{dq}")
                nc.sync.dma_start(xr[:], xTkv[ts(dq, 128), ts(kb, 512)])
                xrs.append(xr)
            for cc in range(2):
                kp_ps = psA.tile([128, 512], dt.float32, tag="psA",
                                 name=f"kp{kb}{cc}")
                for dq in range(DQC):
                    nc.tensor.matmul(kp_ps[:], wk_r[:, dq, ts(cc, 128)],
                                     xrs[dq][:],
                                     start=(dq == 0), stop=(dq == DQC - 1))
                nc.vector.tensor_copy(kT_sb[cc][kb][:], kp_ps[:])
            for half in range(2):
                vp = psA.tile([128, 512], dt.float32, tag="psA",
                              name=f"vp{kb}{half}")
                for dq in range(DQC):
                    for t2 in range(2):
                        t = half * 2 + t2
                        # start clears pending-write state for the whole 2KB
                        # psum bank: only its first matmul may set it
                        nc.tensor.matmul(vp[:, ts(t2, 256)],
                                         xrs[dq][:, ts(t, 128)],
                                         wv_r[:, dq, :],
                                         start=(dq == 0 and t2 == 0),
                                         stop=(dq == DQC - 1))
                for t2 in range(2):
                    t = half * 2 + t2
                    kt = kb * 4 + t
                    va = va_sb[kt]
                    src = vp[:, ts(t2, 256)].rearrange("p (h c) -> p h c", h=HC)
                    nc.vector.tensor_scalar_mul(va[:, :, 0:64], src,
                                                keep_f[:, kt, 0:1])
                    nc.vector.tensor_copy(va[:, :, 64:65], keep_f[:, kt, :])
            attn_kts(0, 0, range(kb * 4, kb * 4 + 4), final=(kb == KB - 1))

        # projections done: psA's banks become a third score buffer
        psA_cm.__exit__(None, None, None)
        spX = ctx.enter_context(tc.tile_pool(name="spX", bufs=1, space="PSUM"))
        sp_pools.append(spX)

        wo_r = load_w3("wor", wo, 2)            # [128, 2, 1024]

        attn_norm(0, 0)

        # ---- remaining attention combos ---------------------------------
        for qb, hp in [(0, 1), (1, 0), (1, 1)]:
            open_oP(qb, hp)
            attn_kts(qb, hp, range(KT), final=True)
            attn_norm(qb, hp)

        # ---- output projection ------------------------------------------
        for qt in range(8):
            for eb in range(2):
                op = psO.tile([128, 512], dt.float32, tag="oP",
                              name=f"op{qt}_{eb}")
                for cc in range(2):
                    nc.tensor.matmul(op[:, :], onT_sb[cc][:, ts(qt, 128)],
                                     wo_r[:, cc, ts(eb, 512)],
                                     start=(cc == 0), stop=(cc == 1))
                osb = outsb_pool.tile([128, 512], dt.float32, tag="osb",
                                      name=f"osb{qt}_{eb}")
                nc.vector.tensor_copy(osb[:], op[:])
                nc.sync.dma_start(out[ts(qt, 128), ts(eb, 512)], osb[:])

        lp.__exit__(None, None, None)


def kernel(x_q, x_kv, pad_mask, Wq, bq, Wk, bk, Wv, bv, Wo, bo):
    global LAST_EXEC_NS
    x_q = np.asarray(x_q, np.float32)
    x_kv = np.asarray(x_kv, np.float32)
    pad_mask = np.asarray(pad_mask)
    Wq, bq = np.asarray(Wq, np.float32), np.asarray(bq, np.float32)
    Wk, bk = np.asarray(Wk, np.float32), np.asarray(bk, np.float32)
    Wv, bv = np.asarray(Wv, np.float32), np.asarray(bv, np.float32)
    Wo, bo = np.asarray(Wo, np.float32), np.asarray(bo, np.float32)

    if "nc" not in _cache:
        _cache["nc"] = _build()
    nc = _cache["nc"]

    Wq_s = (Wq * SCALE).astype(np.float32)
    bq_s = (bq * SCALE).astype(np.float32)

    xTq = [np.ascontiguousarray(x_q[b].T) for b in range(B)]
    xTkv = [np.ascontiguousarray(x_kv[b].T) for b in range(B)]
    keepm = []
    for b in range(B):
        k01 = (~pad_mask[b]).astype(np.float32)          # (L,) 1=keep
        k4 = np.repeat(k01[:, None], HC, axis=1)          # (L, HC)
        keepm.append(np.ascontiguousarray(
            k4.reshape(KT, 128, HC).transpose(1, 0, 2)))  # (128, KT, HC)

    in_maps = []
    for c in range(N_CORES):
        b, g = c // 4, c % 4
        hs = g * CS
        in_maps.append({
            "xTq": xTq[b],
            "xTkv": xTkv[b],
            "wq": np.ascontiguousarray(Wq_s[:, hs:hs + CS]),
            "wk": np.ascontiguousarray(Wk[:, hs:hs + CS]),
            "wv": np.ascontiguousarray(Wv[:, hs:hs + CS]),
            "wo": np.ascontiguousarray(Wo[hs:hs + CS, :]),
            "bqp": np.ascontiguousarray(
                bq_s[hs:hs + CS].reshape(2, 128).T),      # [128, 2] cc-major
            "keep": keepm[b],
        })

    res = run_bass_kernel_spmd(nc, in_maps, list(range(N_CORES)), trace=TRACE)
    LAST_EXEC_NS = res.exec_time_ns

    outp = np.zeros((B, N, D), np.float32)
    for c in range(N_CORES):
        outp[c // 4] += res.results[c]["out"]
    outp += bo + bv @ Wo
    return outp
